# revision 35
# baseline (speedup 1.0000x reference)
"""Trainium2 Bass kernel for nn_Concat_Linear (feat [65536,2,768] -> out [65536,9]).

Data-parallel across 8 NeuronCores (8192 rows each). Per core:

  - the host rounds feat to bf16; rows are DMA'd contiguously (3KB runs, 1024
    descriptors per 1024-row buffer): partition p of group g holds row
    g*128+p. This halves HBM traffic and keeps the DMA engines at line rate
    instead of the descriptor-overhead regime of a scrambled
    feature-on-partition load.
  - the PE transposes X (bf16: 1 cycle/row + fast weight load) into
    feature-on-partition tiles; DVE/ACT copy them from PSUM to SBUF in
    [128, 1024] batches (two chunks per PSUM bank), upcasting to f32r, and
    the projection runs as 2x12 K=128 f32r accumulating matmuls against
    fp32-precision weights (so only X's bf16 rounding costs accuracy).
  - the projection weights fold most of the epilogue: Y rows 0:81 hold
    this[a] replicated 9x (a*9+k), so the trilinear product g*thisbc needs no
    e9-broadcast matmul, and the final linear's l1 term is applied as
    l1e[a*9+k,i] = l1[a,i]/9 against the same rows; Y rows 96:105 hold last.
  - the trilinear form, LayerNorm and final linear run via small PE matmuls +
    DVE/ACT elementwise ops, emitted as 11 phases (both 512-wide halves per
    phase) interleaved between the next buffer's dense transpose/projection
    units so the per-hop chain latency hides under independent work; the
    variance reducer is a full 9x9/9 matrix so reciprocal/sqrt run at
    [9, 512] and no rstd broadcast matmul is needed; rstd =
    Sqrt(reciprocal_approx(var)) keeps ACT on a single activation table
    (min row variance ~4.5e-3 >> eps=1e-5, so eps is numerically irrelevant;
    verified against the reference).
  - outputs accumulate as [9, b_core] (feature-major) and store with 9 long
    descriptors per buffer; the host transposes the tiny [9, b_core] result.
"""

import sys
import types

import numpy as np

B_FULL = 65536
N_CORES = 8
B_CORE = B_FULL // N_CORES
D = 1536  # 2 * 768
NB = 1024  # rows per buffer
G = NB // 128         # row groups per buffer (8)
KB = D // 128         # feature blocks (12)
HB = 512              # epilogue half width
LN_EPS = 1e-5


def _ensure_axon_hooks():
    """Register the NTFF profile hook if the image's antenv lacks axon_hooks.

    Without this, trace=True degrades to no profiling (runs still work)."""
    try:
        import antenv  # noqa: F401
        from antenv import axon_hooks  # noqa: F401
        return
    except ImportError:
        pass
    try:
        import antenv
        mod = types.ModuleType("antenv.axon_hooks")
        mod._hook = None
        mod.set_axon_ntff_profile_hook = lambda h: setattr(mod, "_hook", h)
        mod.get_axon_ntff_profile_hook = lambda: mod._hook
        sys.modules["antenv.axon_hooks"] = mod
        antenv.axon_hooks = mod
        from trn_agent_boot.trn_boot import _ntff_profile_via_ctypes
        mod.set_axon_ntff_profile_hook(
            _ntff_profile_via_ctypes("/opt/axon/libaxon_pjrt.so")
        )
    except Exception:
        pass


def make_consts(W_int, W_stim, trans, ln_w, ln_b, W_out, b_out):
    """Host-side constant tensors."""
    import ml_dtypes
    bf16 = ml_dtypes.bfloat16

    W_int = np.asarray(W_int, np.float32)
    W_stim = np.asarray(W_stim, np.float32)
    trans = np.asarray(trans, np.float32)
    ln_w = np.asarray(ln_w, np.float32)
    ln_b = np.asarray(ln_b, np.float32)
    W_out = np.asarray(W_out, np.float32)
    b_out = np.asarray(b_out, np.float32)

    # Projection weights, Y = W_cat2^T @ x per column (row of the batch):
    #   Y[a*9+k]  = this[a] for k=0..8 (replicated)   (cols 0:81, base 0 so
    #               DVE may span all 81 partitions)
    #   Y[81:90]  = l1^T @ this  (l1 folded into W)   (cols 81:90)
    #   Y[96:105] = last = W_int @ feat[:,0,:]        (cols 96:105,
    #               quadrant-aligned for the g matmul)
    # so the epilogue needs no e9-broadcast matmul and no l1 matmul.
    l1 = np.ascontiguousarray(W_out[:, 0:9].T)
    # l1 applied against the replicated thisbc rows: l1e[a*9+k, i] = l1[a,i]/9
    l1e = np.repeat(l1, 9, axis=0) / 9.0
    W_cat = np.zeros((D, 105), np.float32)
    for a in range(9):
        W_cat[768:1536, a * 9:(a + 1) * 9] = \
            np.repeat(W_stim.T[:, a:a + 1], 9, axis=1)
    W_cat[0:768, 96:105] = W_int.T
    # k-blocked: wc[p, kb, m] = W_cat[kb*128 + p, m]
    wc = np.ascontiguousarray(
        W_cat.reshape(KB, 128, 105).transpose(1, 0, 2))

    # trans matrix for G[a*9+k, b] = sum_j trans[a,j,k] * last[j, b]
    # rows live at partitions 96:105 to match last's position in Y
    # (matmul operands must share their base partition).
    tm = np.zeros((105, 81), np.float32)
    for a in range(9):
        for j in range(9):
            for k in range(9):
                tm[96 + j, a * 9 + k] = trans[a, j, k]

    # bil_centered[k', b] = sum_a M[a*9+k', b] - (1/9) sum_rows M[row, b]
    rp = np.full((81, 9), -1.0 / 9.0, np.float32)
    for a in range(9):
        for k in range(9):
            rp[a * 9 + k, k] += 1.0

    # mean-of-squares reducer, replicated to 9 output partitions so the
    # reciprocal/sqrt/ln chain runs directly at [9, HB] (no rstd broadcast)
    o99 = np.full((9, 9), 1.0 / 9.0, np.float32)

    # Final linear with ln_w/ln_b folded in:
    # out = (l1-fold rows of Y) + (W_out[:, 9:] * ln_w) @ (bil_c * rstd) + b'
    l2 = np.ascontiguousarray((W_out[:, 9:18] * ln_w[None, :]).T)
    bout = (b_out + W_out[:, 9:18] @ ln_b).reshape(9, 1).astype(np.float32)

    i128 = np.eye(128, dtype=bf16)

    return {
        "wc": wc, "tm": tm, "rp": rp, "o99": o99,
        "l1e": l1e, "l2": l2, "bout": bout, "i128": i128,
    }


def build_program(b_core=B_CORE, num_devices=N_CORES):
    import concourse.bass as bass  # noqa: F401
    import concourse.tile as tile
    from concourse import bacc, mybir

    f32 = mybir.dt.float32
    f32r = mybir.dt.float32r
    bf16 = mybir.dt.bfloat16
    nc = bacc.Bacc("TRN2", target_bir_lowering=False, debug=False,
                   num_devices=num_devices)

    feat_d = nc.dram_tensor("feat", [b_core, D], bf16, kind="ExternalInput")
    out_d = nc.dram_tensor("out", [9, b_core], f32, kind="ExternalOutput")
    cshapes = {
        "wc": [128, KB, 105], "tm": [105, 81], "rp": [81, 9],
        "o99": [9, 9], "l1e": [81, 9], "l2": [9, 9],
        "bout": [9, 1], "i128": [128, 128],
    }
    cddt = {"wc": f32r, "i128": bf16, "tm": f32r, "rp": f32r,
            "o99": f32r, "l1e": f32r, "l2": f32r,
            "bout": f32}
    cd = {k: nc.dram_tensor(k, v, cddt[k], kind="ExternalInput")
          for k, v in cshapes.items()}

    nbuf = b_core // NB
    with tile.TileContext(nc) as tc:
        with tc.tile_pool(name="consts", bufs=1) as cp, \
             tc.tile_pool(name="tin", bufs=3) as tinp, \
             tc.tile_pool(name="xt", bufs=8) as xtp, \
             tc.tile_pool(name="ysb", bufs=4) as ysbp, \
             tc.tile_pool(name="episb", bufs=8) as esbp, \
             tc.tile_pool(name="tps", bufs=3, space="PSUM") as tpp, \
             tc.tile_pool(name="yps", bufs=1, space="PSUM") as ypp, \
             tc.tile_pool(name="gps", bufs=2, space="PSUM") as gpp, \
             tc.tile_pool(name="epips", bufs=2, space="PSUM") as epp:

            cs = {k: cp.tile(v, cddt[k], tag=k, name=k)
                  for k, v in cshapes.items()}
            corder = ["i128", "wc"] + [k for k in cshapes
                                       if k not in ("i128", "wc")]
            for k in corder:
                # consts load on the idle gpsimd (SWDGE) queue so both HWDGE
                # queues can stream buffer 0's feat pieces immediately
                nc.gpsimd.dma_start(cs[k][:], cd[k].ap())

            def emit_load(ib, split=2, dual=False):
                t_in = tinp.tile([128, G, D], bf16, tag="t_in",
                                 name=f"t_in{ib}")
                gs = G // split
                for s in range(split):
                    rows = feat_d.ap()[ib * NB + s * gs * 128:
                                       ib * NB + (s + 1) * gs * 128, :]
                    eng = nc.scalar if (dual and s % 2) else nc.sync
                    eng.dma_start(
                        t_in[:, s * gs:(s + 1) * gs, :],
                        rows.rearrange("(g p) d -> p g d", g=gs, p=128))
                return t_in

            def make_epi_phases(ib, y_sbs):
                """Epilogue for buffer ib as phase closures; each phase does
                both 512-wide halves (two independent chains), interleaved
                with the next buffer's dense units at 2-unit spacing."""
                st = {}

                def ph_g():
                    for h in (0, 1):
                        gp = gpp.tile([81, HB], f32, tag="g",
                                      name=f"g{ib}_{h}")
                        nc.tensor.matmul(gp[:], cs["tm"][96:105, :],
                                         y_sbs[h][96:105, :],
                                         tile_position=(96, 0))
                        st[f"g{h}"] = gp

                def ph_m():
                    for h in (0, 1):
                        m = esbp.tile([81, HB], f32r, tag="ep_sb",
                                      name=f"m{ib}_{h}")
                        nc.vector.tensor_mul(m[:], st[f"g{h}"][:],
                                             y_sbs[h][0:81, :])
                        st[f"m{h}"] = m

                def ph_bil():
                    for h in (0, 1):
                        bp = epp.tile([9, HB], f32, tag="eps",
                                      name=f"bil{ib}_{h}")
                        nc.tensor.matmul(bp[:], cs["rp"][:], st[f"m{h}"][:])
                        st[f"bilp{h}"] = bp

                def ph_bilcopy():
                    for h in (0, 1):
                        bs = esbp.tile([9, HB], f32, tag="ep_sb",
                                       name=f"bils{ib}_{h}")
                        nc.scalar.copy(bs[:], st[f"bilp{h}"][:])
                        st[f"bils{h}"] = bs

                def ph_sq():
                    for h in (0, 1):
                        sq = esbp.tile([9, HB], f32r, tag="ep_sb",
                                       name=f"sq{ib}_{h}")
                        nc.vector.tensor_mul(sq[:], st[f"bils{h}"][:],
                                             st[f"bils{h}"][:])
                        st[f"sq{h}"] = sq

                def ph_var():
                    for h in (0, 1):
                        vp = epp.tile([9, HB], f32, tag="eps",
                                      name=f"var{ib}_{h}")
                        nc.tensor.matmul(vp[:], cs["o99"][:], st[f"sq{h}"][:])
                        st[f"var{h}"] = vp

                def ph_recip():
                    for h in (0, 1):
                        vr = esbp.tile([9, HB], f32, tag="ep_sb",
                                       name=f"vrec{ib}_{h}")
                        nc.vector.reciprocal_approx_fast(vr[:],
                                                         st[f"var{h}"][:])
                        st[f"vrec{h}"] = vr

                def ph_sqrt():
                    for h in (0, 1):
                        rs = esbp.tile([9, HB], f32r, tag="ep_sb",
                                       name=f"rstd{ib}_{h}")
                        nc.scalar.activation(
                            rs[:], st[f"vrec{h}"][:],
                            mybir.ActivationFunctionType.Sqrt)
                        st[f"rstd{h}"] = rs

                def ph_ln():
                    for h in (0, 1):
                        lns = esbp.tile([9, HB], f32r, tag="ep_sb",
                                        name=f"lns{ib}_{h}")
                        nc.vector.tensor_mul(lns[:], st[f"rstd{h}"][:],
                                             st[f"bils{h}"][:])
                        st[f"ln{h}"] = lns

                def ph_o():
                    for h in (0, 1):
                        op = epp.tile([9, HB], f32, tag="eps",
                                      name=f"o{ib}_{h}")
                        nc.tensor.matmul(op[:], cs["l2"][:], st[f"ln{h}"][:],
                                         start=True, stop=False)
                        nc.tensor.matmul(op[:], cs["l1e"][:],
                                         y_sbs[h][0:81, :],
                                         start=False, stop=True)
                        st[f"o{h}"] = op

                def ph_add():
                    osb = esbp.tile([9, NB], f32, tag="osb", name=f"osb{ib}")
                    for h in (0, 1):
                        nc.vector.tensor_scalar_add(
                            osb[:, h * HB:(h + 1) * HB], st[f"o{h}"][:],
                            cs["bout"][:, 0:1])
                    # feature-major store: 9 descriptors of 4KB
                    nc.scalar.dma_start(
                        out_d.ap()[:, ib * NB:(ib + 1) * NB], osb[:])

                return [ph_g, ph_m, ph_bil, ph_bilcopy,
                        ph_sq, ph_var, ph_recip, ph_sqrt, ph_ln,
                        ph_o, ph_add]

            def emit_proj(ib, t_in, phases):
                """Transpose X to feature-major, project, and interleave the
                previous buffer's epilogue phases between dense units."""
                phases = list(phases)
                unit = 0

                def tick():
                    nonlocal unit
                    unit += 1
                    if unit % 2 == 0 and phases:
                        phases.pop(0)()

                xtp_pairs = [xtp.tile([128, 2, NB], f32r, tag="xt",
                                      name=f"xt{ib}_{kbp}")
                             for kbp in range(KB // 2)]
                xts = [xtp_pairs[kb // 2][:, kb % 2, :] for kb in range(KB)]
                y_sbs = []
                for h in (0, 1):
                    for kbp in range(KB // 2):
                        # one PSUM bank holds two chunks (8 transposes), so
                        # each PSUM->SBUF upcast copy moves [128, 1024]
                        tp = tpp.tile([128, 2, 4, 128], bf16, tag="tp",
                                      name=f"tp{ib}_{h}_{kbp}")
                        for sub in (0, 1):
                            kb = kbp * 2 + sub
                            for gg in range(4):
                                g = h * 4 + gg
                                nc.tensor.matmul(
                                    tp[:, sub, gg, :],
                                    t_in[:, g, kb * 128:(kb + 1) * 128],
                                    cs["i128"][:],
                                    is_transpose=True,
                                    start=(gg == 0), stop=(gg == 3),
                                )
                            tick()
                        dst = xtp_pairs[kbp][:, :, h * HB:(h + 1) * HB]
                        src_v = tp[:].rearrange("p s gg c -> p s (gg c)")
                        # split PSUM->SBUF upcast copies across DVE and ACT
                        if kbp % 2 == 0:
                            nc.vector.tensor_copy(dst, src_v)
                        else:
                            nc.scalar.copy(dst, src_v)
                    y_ps = ypp.tile([105, HB], f32, tag="y", name=f"y{ib}_{h}")
                    for kb in range(KB):
                        nc.tensor.matmul(
                            y_ps[:],
                            cs["wc"][:, kb, :],
                            xts[kb][:, h * HB:(h + 1) * HB],
                            start=(kb == 0), stop=(kb == KB - 1),
                        )
                    y_sb = ysbp.tile([105, HB], f32r, tag="y_sb",
                                     name=f"ysb{ib}_{h}")
                    nc.scalar.copy(y_sb[:], y_ps[:])
                    y_sbs.append(y_sb)
                    tick()
                for ph in phases:
                    ph()
                return y_sbs

            tins = [emit_load(0, split=8, dual=True), emit_load(1)]
            phases = []
            for ib in range(nbuf):
                if ib + 2 < nbuf:
                    tins.append(emit_load(ib + 2))
                y_sbs = emit_proj(ib, tins[ib], phases)
                phases = make_epi_phases(ib, y_sbs)
            for ph in phases:
                ph()
    nc.compile()
    return nc


_PROGRAM = None


def _get_program():
    global _PROGRAM
    if _PROGRAM is None:
        _PROGRAM = build_program()
    return _PROGRAM


def kernel(feat, W_int, W_stim, trans, ln_w, ln_b, W_out, b_out,
           trace=False, trace_kwargs=None):
    _ensure_axon_hooks()
    from concourse.bass_utils import run_bass_kernel_spmd
    import ml_dtypes

    feat = np.asarray(feat, np.float32)
    feat2 = feat.reshape(B_FULL, D).astype(ml_dtypes.bfloat16)
    consts = make_consts(W_int, W_stim, trans, ln_w, ln_b, W_out, b_out)
    nc = _get_program()
    in_maps = []
    for c in range(N_CORES):
        m = {"feat": np.ascontiguousarray(feat2[c * B_CORE:(c + 1) * B_CORE])}
        m.update(consts)
        in_maps.append(m)
    res = run_bass_kernel_spmd(nc, in_maps, list(range(N_CORES)), trace=trace)
    out = np.concatenate(
        [res.results[c]["out"].T for c in range(N_CORES)], axis=0)
    kernel.last_results = res
    return np.ascontiguousarray(out, dtype=np.float32)


# revision 36
# speedup vs baseline: 1.0077x; 1.0077x over previous
"""Trainium2 Bass kernel for nn_Concat_Linear (feat [65536,2,768] -> out [65536,9]).

Data-parallel across 8 NeuronCores (8192 rows each). Per core:

  - the host rounds feat to bf16; rows are DMA'd contiguously (3KB runs, 1024
    descriptors per 1024-row buffer): partition p of group g holds row
    g*128+p. This halves HBM traffic and keeps the DMA engines at line rate
    instead of the descriptor-overhead regime of a scrambled
    feature-on-partition load.
  - the PE transposes X (bf16: 1 cycle/row + fast weight load) into
    feature-on-partition tiles; DVE/ACT copy them from PSUM to SBUF in
    [128, 1024] batches (two chunks per PSUM bank), upcasting to f32r, and
    the projection runs as 2x12 K=128 f32r accumulating matmuls against
    fp32-precision weights (so only X's bf16 rounding costs accuracy).
  - the projection weights fold most of the epilogue: Y rows 0:81 hold
    this[a] replicated 9x (a*9+k), so the trilinear product g*thisbc needs no
    e9-broadcast matmul, and the final linear's l1 term is applied as
    l1e[a*9+k,i] = l1[a,i]/9 against the same rows; Y rows 96:105 hold last.
  - the trilinear form, LayerNorm and final linear run via small PE matmuls +
    DVE/ACT elementwise ops, emitted as 11 phases (both 512-wide halves per
    phase) interleaved between the next buffer's dense transpose/projection
    units so the per-hop chain latency hides under independent work; the
    variance reducer is a full 9x9/9 matrix so reciprocal/sqrt run at
    [9, 512] and no rstd broadcast matmul is needed; rstd =
    Sqrt(reciprocal_approx(var)) keeps ACT on a single activation table
    (min row variance ~4.5e-3 >> eps=1e-5, so eps is numerically irrelevant;
    verified against the reference).
  - outputs accumulate as [9, b_core] (feature-major) and store with 9 long
    descriptors per buffer; the host transposes the tiny [9, b_core] result.
"""

import sys
import types

import numpy as np

B_FULL = 65536
N_CORES = 8
B_CORE = B_FULL // N_CORES
D = 1536  # 2 * 768
NB = 1024  # rows per buffer
G = NB // 128         # row groups per buffer (8)
KB = D // 128         # feature blocks (12)
HB = 512              # epilogue half width
LN_EPS = 1e-5


def _ensure_axon_hooks():
    """Register the NTFF profile hook if the image's antenv lacks axon_hooks.

    Without this, trace=True degrades to no profiling (runs still work)."""
    try:
        import antenv  # noqa: F401
        from antenv import axon_hooks  # noqa: F401
        return
    except ImportError:
        pass
    try:
        import antenv
        mod = types.ModuleType("antenv.axon_hooks")
        mod._hook = None
        mod.set_axon_ntff_profile_hook = lambda h: setattr(mod, "_hook", h)
        mod.get_axon_ntff_profile_hook = lambda: mod._hook
        sys.modules["antenv.axon_hooks"] = mod
        antenv.axon_hooks = mod
        from trn_agent_boot.trn_boot import _ntff_profile_via_ctypes
        mod.set_axon_ntff_profile_hook(
            _ntff_profile_via_ctypes("/opt/axon/libaxon_pjrt.so")
        )
    except Exception:
        pass


def make_consts(W_int, W_stim, trans, ln_w, ln_b, W_out, b_out):
    """Host-side constant tensors."""
    import ml_dtypes
    bf16 = ml_dtypes.bfloat16

    W_int = np.asarray(W_int, np.float32)
    W_stim = np.asarray(W_stim, np.float32)
    trans = np.asarray(trans, np.float32)
    ln_w = np.asarray(ln_w, np.float32)
    ln_b = np.asarray(ln_b, np.float32)
    W_out = np.asarray(W_out, np.float32)
    b_out = np.asarray(b_out, np.float32)

    # Projection weights, Y = W_cat2^T @ x per column (row of the batch):
    #   Y[a*9+k]  = this[a] for k=0..8 (replicated)   (cols 0:81, base 0 so
    #               DVE may span all 81 partitions)
    #   Y[81:90]  = l1^T @ this  (l1 folded into W)   (cols 81:90)
    #   Y[96:105] = last = W_int @ feat[:,0,:]        (cols 96:105,
    #               quadrant-aligned for the g matmul)
    # so the epilogue needs no e9-broadcast matmul and no l1 matmul.
    l1 = np.ascontiguousarray(W_out[:, 0:9].T)
    # l1 applied against the replicated thisbc rows: l1e[a*9+k, i] = l1[a,i]/9
    l1e = np.repeat(l1, 9, axis=0) / 9.0
    W_cat = np.zeros((D, 105), np.float32)
    for a in range(9):
        W_cat[768:1536, a * 9:(a + 1) * 9] = \
            np.repeat(W_stim.T[:, a:a + 1], 9, axis=1)
    W_cat[0:768, 96:105] = W_int.T
    # k-blocked: wc[p, kb, m] = W_cat[kb*128 + p, m]
    wc = np.ascontiguousarray(
        W_cat.reshape(KB, 128, 105).transpose(1, 0, 2))

    # trans matrix for G[a*9+k, b] = sum_j trans[a,j,k] * last[j, b]
    # rows live at partitions 96:105 to match last's position in Y
    # (matmul operands must share their base partition).
    tm = np.zeros((105, 81), np.float32)
    for a in range(9):
        for j in range(9):
            for k in range(9):
                tm[96 + j, a * 9 + k] = trans[a, j, k]

    # bil_centered[k', b] = sum_a M[a*9+k', b] - (1/9) sum_rows M[row, b]
    rp = np.full((81, 9), -1.0 / 9.0, np.float32)
    for a in range(9):
        for k in range(9):
            rp[a * 9 + k, k] += 1.0

    # mean-of-squares reducer, replicated to 9 output partitions so the
    # reciprocal/sqrt/ln chain runs directly at [9, HB] (no rstd broadcast)
    o99 = np.full((9, 9), 1.0 / 9.0, np.float32)

    # Final linear with ln_w/ln_b folded in:
    # out = (l1-fold rows of Y) + (W_out[:, 9:] * ln_w) @ (bil_c * rstd) + b'
    l2 = np.ascontiguousarray((W_out[:, 9:18] * ln_w[None, :]).T)
    bout = (b_out + W_out[:, 9:18] @ ln_b).reshape(9, 1).astype(np.float32)

    i128 = np.eye(128, dtype=bf16)

    return {
        "wc": wc, "tm": tm, "rp": rp, "o99": o99,
        "l1e": l1e, "l2": l2, "bout": bout, "i128": i128,
    }


def build_program(b_core=B_CORE, num_devices=N_CORES):
    import concourse.bass as bass  # noqa: F401
    import concourse.tile as tile
    from concourse import bacc, mybir

    f32 = mybir.dt.float32
    f32r = mybir.dt.float32r
    bf16 = mybir.dt.bfloat16
    nc = bacc.Bacc("TRN2", target_bir_lowering=False, debug=False,
                   num_devices=num_devices)

    feat_d = nc.dram_tensor("feat", [b_core, D], bf16, kind="ExternalInput")
    out_d = nc.dram_tensor("out", [9, b_core], f32, kind="ExternalOutput")
    cshapes = {
        "wc": [128, KB, 105], "tm": [105, 81], "rp": [81, 9],
        "o99": [9, 9], "l1e": [81, 9], "l2": [9, 9],
        "bout": [9, 1], "i128": [128, 128],
    }
    cddt = {"wc": f32r, "i128": bf16, "tm": f32r, "rp": f32r,
            "o99": f32r, "l1e": f32r, "l2": f32r,
            "bout": f32}
    cd = {k: nc.dram_tensor(k, v, cddt[k], kind="ExternalInput")
          for k, v in cshapes.items()}

    nbuf = b_core // NB
    with tile.TileContext(nc) as tc:
        with tc.tile_pool(name="consts", bufs=1) as cp, \
             tc.tile_pool(name="tin", bufs=3) as tinp, \
             tc.tile_pool(name="xt", bufs=8) as xtp, \
             tc.tile_pool(name="ysb", bufs=4) as ysbp, \
             tc.tile_pool(name="episb", bufs=8) as esbp, \
             tc.tile_pool(name="tps", bufs=3, space="PSUM") as tpp, \
             tc.tile_pool(name="yps", bufs=1, space="PSUM") as ypp, \
             tc.tile_pool(name="gps", bufs=2, space="PSUM") as gpp, \
             tc.tile_pool(name="epips", bufs=2, space="PSUM") as epp:

            cs = {k: cp.tile(v, cddt[k], tag=k, name=k)
                  for k, v in cshapes.items()}
            corder = ["i128", "wc"] + [k for k in cshapes
                                       if k not in ("i128", "wc")]
            for k in corder:
                # consts load on the ACT HWDGE queue so the SP queue can
                # start streaming feat immediately
                nc.scalar.dma_start(cs[k][:], cd[k].ap())

            def emit_load(ib, split=2, dual=False):
                t_in = tinp.tile([128, G, D], bf16, tag="t_in",
                                 name=f"t_in{ib}")
                gs = G // split
                for s in range(split):
                    rows = feat_d.ap()[ib * NB + s * gs * 128:
                                       ib * NB + (s + 1) * gs * 128, :]
                    eng = nc.scalar if (dual and s % 2) else nc.sync
                    eng.dma_start(
                        t_in[:, s * gs:(s + 1) * gs, :],
                        rows.rearrange("(g p) d -> p g d", g=gs, p=128))
                return t_in

            def make_epi_phases(ib, y_sbs):
                """Epilogue for buffer ib as phase closures; each phase does
                both 512-wide halves (two independent chains), interleaved
                with the next buffer's dense units at 2-unit spacing."""
                st = {}

                def ph_g():
                    for h in (0, 1):
                        gp = gpp.tile([81, HB], f32, tag="g",
                                      name=f"g{ib}_{h}")
                        nc.tensor.matmul(gp[:], cs["tm"][96:105, :],
                                         y_sbs[h][96:105, :],
                                         tile_position=(96, 0))
                        st[f"g{h}"] = gp

                def ph_m():
                    for h in (0, 1):
                        m = esbp.tile([81, HB], f32r, tag="ep_sb",
                                      name=f"m{ib}_{h}")
                        nc.vector.tensor_mul(m[:], st[f"g{h}"][:],
                                             y_sbs[h][0:81, :])
                        st[f"m{h}"] = m

                def ph_bil():
                    for h in (0, 1):
                        bp = epp.tile([9, HB], f32, tag="eps",
                                      name=f"bil{ib}_{h}")
                        nc.tensor.matmul(bp[:], cs["rp"][:], st[f"m{h}"][:])
                        st[f"bilp{h}"] = bp

                def ph_bilcopy():
                    for h in (0, 1):
                        bs = esbp.tile([9, HB], f32, tag="ep_sb",
                                       name=f"bils{ib}_{h}")
                        nc.scalar.copy(bs[:], st[f"bilp{h}"][:])
                        st[f"bils{h}"] = bs

                def ph_sq():
                    for h in (0, 1):
                        sq = esbp.tile([9, HB], f32r, tag="ep_sb",
                                       name=f"sq{ib}_{h}")
                        nc.vector.tensor_mul(sq[:], st[f"bils{h}"][:],
                                             st[f"bils{h}"][:])
                        st[f"sq{h}"] = sq

                def ph_var():
                    for h in (0, 1):
                        vp = epp.tile([9, HB], f32, tag="eps",
                                      name=f"var{ib}_{h}")
                        nc.tensor.matmul(vp[:], cs["o99"][:], st[f"sq{h}"][:])
                        st[f"var{h}"] = vp

                def ph_recip():
                    for h in (0, 1):
                        vr = esbp.tile([9, HB], f32, tag="ep_sb",
                                       name=f"vrec{ib}_{h}")
                        nc.vector.reciprocal_approx_fast(vr[:],
                                                         st[f"var{h}"][:])
                        st[f"vrec{h}"] = vr

                def ph_sqrt():
                    for h in (0, 1):
                        rs = esbp.tile([9, HB], f32r, tag="ep_sb",
                                       name=f"rstd{ib}_{h}")
                        nc.scalar.activation(
                            rs[:], st[f"vrec{h}"][:],
                            mybir.ActivationFunctionType.Sqrt)
                        st[f"rstd{h}"] = rs

                def ph_ln():
                    for h in (0, 1):
                        lns = esbp.tile([9, HB], f32r, tag="ep_sb",
                                        name=f"lns{ib}_{h}")
                        nc.vector.tensor_mul(lns[:], st[f"rstd{h}"][:],
                                             st[f"bils{h}"][:])
                        st[f"ln{h}"] = lns

                def ph_o():
                    for h in (0, 1):
                        op = epp.tile([9, HB], f32, tag="eps",
                                      name=f"o{ib}_{h}")
                        nc.tensor.matmul(op[:], cs["l2"][:], st[f"ln{h}"][:],
                                         start=True, stop=False)
                        nc.tensor.matmul(op[:], cs["l1e"][:],
                                         y_sbs[h][0:81, :],
                                         start=False, stop=True)
                        st[f"o{h}"] = op

                def ph_add():
                    osb = esbp.tile([9, NB], f32, tag="osb", name=f"osb{ib}")
                    for h in (0, 1):
                        nc.vector.tensor_scalar_add(
                            osb[:, h * HB:(h + 1) * HB], st[f"o{h}"][:],
                            cs["bout"][:, 0:1])
                    # feature-major store: 9 descriptors of 4KB
                    nc.scalar.dma_start(
                        out_d.ap()[:, ib * NB:(ib + 1) * NB], osb[:])

                return [ph_g, ph_m, ph_bil, ph_bilcopy,
                        ph_sq, ph_var, ph_recip, ph_sqrt, ph_ln,
                        ph_o, ph_add]

            def emit_proj(ib, t_in, phases):
                """Transpose X to feature-major, project, and interleave the
                previous buffer's epilogue phases between dense units."""
                phases = list(phases)
                unit = 0

                def tick():
                    nonlocal unit
                    unit += 1
                    if unit % 2 == 0 and phases:
                        phases.pop(0)()

                xtp_pairs = [xtp.tile([128, 2, NB], f32r, tag="xt",
                                      name=f"xt{ib}_{kbp}")
                             for kbp in range(KB // 2)]
                xts = [xtp_pairs[kb // 2][:, kb % 2, :] for kb in range(KB)]
                y_sbs = []
                for h in (0, 1):
                    for kbp in range(KB // 2):
                        # one PSUM bank holds two chunks (8 transposes), so
                        # each PSUM->SBUF upcast copy moves [128, 1024]
                        tp = tpp.tile([128, 2, 4, 128], bf16, tag="tp",
                                      name=f"tp{ib}_{h}_{kbp}")
                        for sub in (0, 1):
                            kb = kbp * 2 + sub
                            for gg in range(4):
                                g = h * 4 + gg
                                nc.tensor.matmul(
                                    tp[:, sub, gg, :],
                                    t_in[:, g, kb * 128:(kb + 1) * 128],
                                    cs["i128"][:],
                                    is_transpose=True,
                                    start=(gg == 0), stop=(gg == 3),
                                )
                            tick()
                        dst = xtp_pairs[kbp][:, :, h * HB:(h + 1) * HB]
                        src_v = tp[:].rearrange("p s gg c -> p s (gg c)")
                        # split PSUM->SBUF upcast copies across DVE and ACT
                        if kbp % 2 == 0:
                            nc.vector.tensor_copy(dst, src_v)
                        else:
                            nc.scalar.copy(dst, src_v)
                    y_ps = ypp.tile([105, HB], f32, tag="y", name=f"y{ib}_{h}")
                    for kb in range(KB):
                        nc.tensor.matmul(
                            y_ps[:],
                            cs["wc"][:, kb, :],
                            xts[kb][:, h * HB:(h + 1) * HB],
                            start=(kb == 0), stop=(kb == KB - 1),
                        )
                    y_sb = ysbp.tile([105, HB], f32r, tag="y_sb",
                                     name=f"ysb{ib}_{h}")
                    nc.scalar.copy(y_sb[:], y_ps[:])
                    y_sbs.append(y_sb)
                    tick()
                for ph in phases:
                    ph()
                return y_sbs

            tins = [emit_load(0, split=8), emit_load(1)]
            phases = []
            for ib in range(nbuf):
                if ib + 2 < nbuf:
                    tins.append(emit_load(ib + 2))
                y_sbs = emit_proj(ib, tins[ib], phases)
                phases = make_epi_phases(ib, y_sbs)
            for ph in phases:
                ph()
    nc.compile()
    return nc


_PROGRAM = None


def _get_program():
    global _PROGRAM
    if _PROGRAM is None:
        _PROGRAM = build_program()
    return _PROGRAM


def kernel(feat, W_int, W_stim, trans, ln_w, ln_b, W_out, b_out,
           trace=False, trace_kwargs=None):
    _ensure_axon_hooks()
    from concourse.bass_utils import run_bass_kernel_spmd
    import ml_dtypes

    feat = np.asarray(feat, np.float32)
    feat2 = feat.reshape(B_FULL, D).astype(ml_dtypes.bfloat16)
    consts = make_consts(W_int, W_stim, trans, ln_w, ln_b, W_out, b_out)
    nc = _get_program()
    in_maps = []
    for c in range(N_CORES):
        m = {"feat": np.ascontiguousarray(feat2[c * B_CORE:(c + 1) * B_CORE])}
        m.update(consts)
        in_maps.append(m)
    res = run_bass_kernel_spmd(nc, in_maps, list(range(N_CORES)), trace=trace)
    out = np.concatenate(
        [res.results[c]["out"].T for c in range(N_CORES)], axis=0)
    kernel.last_results = res
    return np.ascontiguousarray(out, dtype=np.float32)
